# revision 34
# baseline (speedup 1.0000x reference)
"""Trainium2 Bass kernel for Bengio03HighwayBiLm.

Reference computation (per batch row b, sequence length S, dim D):
  padded = [left_pad(2), x_b(S), right_pad(2)]           # [S+4, D]
  left_win[t]  = padded[t:t+3]   flattened -> [3D]
  right_win[t] = padded[t+2:t+5] flattened -> [3D]
  left  = relu(left_win  @ left_W.T  + left_b)           # [S, D]
  right = relu(right_win @ right_W.T + right_b)
  2x highway on each side: proj = x @ W_l.T + b_l  ([S, 2D])
     x = sigmoid(proj[:,D:]) * x + (1-sigmoid(proj[:,D:])) * relu(proj[:,:D])
  out = concat([left, right], -1)                        # [S, 2D]

Strategy: data-parallel over the batch dim across 8 NeuronCores (4 rows per
core).  On-chip layout is feature-major ([feature partitions, token free dim])
so the window projection becomes 3 shifted GEMMs accumulated in PSUM and the
highway GEMMs chain with no transposes.  The input is transposed on the PE
(128x128 transpose-mode tiles) on the way in, and the final activations are
transposed back on the way out.  Matmuls run in float32r (full-rate 4-byte
mode at N=512).
"""

import numpy as np

import concourse.bass as bass
import concourse.mybir as mybir
import concourse.tile as tile
from concourse import bacc
from concourse.bass_utils import run_bass_kernel_spmd

B, S, D = 32, 512, 512
W = 2
NL = 2
IN_SZ = D * (W + 1)
NCORES = 8
BPC = B // NCORES            # batch rows per core
P = 128
KT_D = D // P                # 4 feature tiles of the model dim
KT_IN = IN_SZ // P           # 12 contraction tiles of the window GEMM
TB = S // P                  # 4 token tiles per row

F32 = mybir.dt.float32
DT = mybir.dt.float32r       # matmul I/O dtype (4-byte, full PE rate at N>=256)

RELU = mybir.ActivationFunctionType.Relu
SIGMOID = mybir.ActivationFunctionType.Sigmoid


def build_nc():
    nc = bacc.Bacc("TRN2", target_bir_lowering=False, debug=False)

    x_d = nc.dram_tensor("x", [BPC, S, D], DT, kind="ExternalInput")
    lWT_d = nc.dram_tensor("lWT", [IN_SZ, D], DT, kind="ExternalInput")
    rWT_d = nc.dram_tensor("rWT", [IN_SZ, D], DT, kind="ExternalInput")
    lhw_d = nc.dram_tensor("lhwWT", [NL, D, 2 * D], DT, kind="ExternalInput")
    rhw_d = nc.dram_tensor("rhwWT", [NL, D, 2 * D], DT, kind="ExternalInput")
    lpad_d = nc.dram_tensor("lpadT", [P, KT_D, W], DT, kind="ExternalInput")
    rpad_d = nc.dram_tensor("rpadT", [P, KT_D, W], DT, kind="ExternalInput")
    lb_d = nc.dram_tensor("lb", [P, KT_D], F32, kind="ExternalInput")
    rb_d = nc.dram_tensor("rb", [P, KT_D], F32, kind="ExternalInput")
    lhwb_d = nc.dram_tensor("lhwb", [P, NL, 2 * KT_D], F32, kind="ExternalInput")
    rhwb_d = nc.dram_tensor("rhwb", [P, NL, 2 * KT_D], F32, kind="ExternalInput")
    ident_d = nc.dram_tensor("ident", [P, P], DT, kind="ExternalInput")
    y_d = nc.dram_tensor("y", [BPC, S, 2 * D], F32, kind="ExternalOutput")

    with tile.TileContext(nc) as tc:
        with (
            tc.tile_pool(name="const", bufs=1) as const,
            tc.tile_pool(name="xtok", bufs=2) as xtok_pool,
            tc.tile_pool(name="xpad", bufs=2) as xpad_pool,
            tc.tile_pool(name="act", bufs=8) as act_pool,
            tc.tile_pool(name="ew", bufs=3) as ew_pool,
            tc.tile_pool(name="yout", bufs=4) as y_pool,
            tc.tile_pool(name="psum", bufs=6, space="PSUM") as psum_pool,
        ):
            ident = const.tile([P, P], DT, name="ident")
            nc.sync.dma_start(out=ident, in_=ident_d[:])

            # weights stream on the scalar HWDGE queue, concurrently with the
            # input loads on the sync queue, in first-use order
            lW_sb = const.tile([P, KT_IN, D], DT, name="lW_sb")
            rW_sb = const.tile([P, KT_IN, D], DT, name="rW_sb")
            lhw_sb = const.tile([P, NL, KT_D, 2 * D], DT, name="lhw_sb")
            rhw_sb = const.tile([P, NL, KT_D, 2 * D], DT, name="rhw_sb")
            lpad_sb = const.tile([P, KT_D, W], DT, name="lpad_sb")
            nc.scalar.dma_start(out=lpad_sb, in_=lpad_d[:])
            rpad_sb = const.tile([P, KT_D, W], DT, name="rpad_sb")
            nc.scalar.dma_start(out=rpad_sb, in_=rpad_d[:])

            def window_gemm(xpads, W_sb, b_sb, j_off, tag):
                """relu(win @ W.T + b): 12 accumulated MMs per output ftile,
                k-outer so each weight k-tile is consumed once, as it lands."""
                pss = [
                    psum_pool.tile([P, S], F32, name="ps_mm", tag="mm", bufs=7)
                    for _ in range(KT_D)
                ]
                for j in range(W + 1):
                    for k in range(KT_D):
                        ki = j * KT_D + k
                        for f in range(KT_D):
                            nc.tensor.matmul(
                                out=pss[f],
                                lhsT=W_sb[:, ki, f * P : (f + 1) * P],
                                rhs=xpads[k][:, j + j_off : j + j_off + S],
                                start=(ki == 0),
                                stop=(ki == KT_IN - 1),
                            )
                out_tiles = []
                for f in range(KT_D):
                    t = act_pool.tile([P, S], DT, name=f"x_{tag}", tag=f"x_{tag}")
                    nc.scalar.activation(
                        out=t, in_=pss[f], func=RELU, bias=b_sb[:, f : f + 1],
                        scale=1.0,
                    )
                    out_tiles.append(t)
                return out_tiles

            def highway_pair(xl_tiles, xr_tiles):
                """Both highway stacks, layer-interleaved across sides so each
                side's ACT/DVE drain chain hides behind the other side's
                matmuls."""
                sides = {"l": (xl_tiles, lhw_sb, lhwb_sb), "r": (xr_tiles, rhw_sb, rhwb_sb)}
                for l in range(NL):
                    for tag in ("l", "r"):
                        x_tiles, hw_sb, hwb_sb = sides[tag]
                        new_x = []
                        for f in range(KT_D):
                            ps_n = psum_pool.tile([P, S], F32, name="ps_mm", tag="mm", bufs=7)
                            for k in range(KT_D):
                                nc.tensor.matmul(
                                    out=ps_n,
                                    lhsT=hw_sb[:, l, k, f * P : (f + 1) * P],
                                    rhs=x_tiles[k],
                                    start=(k == 0),
                                    stop=(k == KT_D - 1),
                                )
                            nl_t = ew_pool.tile([P, S], DT, name="nl_t", tag="nl", bufs=5)
                            nc.scalar.activation(
                                out=nl_t, in_=ps_n, func=RELU,
                                bias=hwb_sb[:, l, f : f + 1], scale=1.0,
                            )
                            ps_g = psum_pool.tile([P, S], F32, name="ps_mm", tag="mm", bufs=7)
                            for k in range(KT_D):
                                nc.tensor.matmul(
                                    out=ps_g,
                                    lhsT=hw_sb[:, l, k, D + f * P : D + (f + 1) * P],
                                    rhs=x_tiles[k],
                                    start=(k == 0),
                                    stop=(k == KT_D - 1),
                                )
                            g_t = ew_pool.tile([P, S], DT, name="g_t", tag="g", bufs=5)
                            nc.scalar.activation(
                                out=g_t, in_=ps_g, func=SIGMOID,
                                bias=hwb_sb[:, l, KT_D + f : KT_D + f + 1], scale=1.0,
                            )
                            # y = g*x + (1-g)*nl = nl + g*(x - nl)
                            d_t = ew_pool.tile([P, S], DT, name="d_t", tag="d", bufs=3)
                            nc.vector.tensor_sub(out=d_t, in0=x_tiles[f], in1=nl_t)
                            nc.vector.tensor_mul(out=d_t, in0=d_t, in1=g_t)
                            xn = act_pool.tile([P, S], DT, name=f"x_{tag}", tag=f"x_{tag}")
                            nc.vector.tensor_add(out=xn, in0=d_t, in1=nl_t)
                            new_x.append(xn)
                        sides[tag] = (new_x, hw_sb, hwb_sb)
                return sides["l"][0], sides["r"][0]

            def load_transpose_row(r, mid_cb=None):
                """DMA row r token-major and PE-transpose into a padded
                feature-major tile [P, KT_D, S+2W]."""
                xp = xpad_pool.tile([P, KT_D, S + 2 * W], DT, name="xp", tag="xp")
                nc.vector.tensor_copy(out=xp[:, :, 0:W], in_=lpad_sb)
                nc.vector.tensor_copy(out=xp[:, :, W + S :], in_=rpad_sb)
                for st in range(TB):
                    x_tok = xtok_pool.tile([P, D], DT, name="x_tok", tag="x_tok")
                    nc.sync.dma_start(out=x_tok, in_=x_d[r, st * P : (st + 1) * P, :])
                    if st == 1 and mid_cb is not None:
                        mid_cb()
                    ps_t = psum_pool.tile([P, KT_D, P], DT, name="ps_tr", tag="tr", bufs=1)
                    for k in range(KT_D):
                        nc.tensor.transpose(
                            out=ps_t[:, k, :], in_=x_tok[:, k * P : (k + 1) * P],
                            identity=ident,
                        )
                    nc.vector.tensor_copy(
                        out=xp[:, :, W + st * P : W + (st + 1) * P], in_=ps_t
                    )
                return [xp[:, k, :] for k in range(KT_D)]

            # 2-deep software pipeline over rows: window GEMMs run one row
            # ahead of the highway stack so the highway-weight DMA stream has
            # PE work to hide behind; input transposes run a row ahead of the
            # window GEMMs.
            def window_row(xpads, mid_cb=None):
                xl = window_gemm(xpads, lW_sb, lb_sb, 0, "l")
                if mid_cb is not None:
                    mid_cb()
                xr = window_gemm(xpads, rW_sb, rb_sb, W, "r")
                return xl, xr

            pend_x = {}   # r -> xpads
            pend_w = {}   # r -> (xl, xr)
            lWT_r = lWT_d[:].rearrange("(kt p) h -> p kt h", p=P)

            def _lw_chunk0():
                nc.sync.dma_start(out=lW_sb[:, 0:2, :], in_=lWT_r[:, 0:2, :])

            pend_x[0] = load_transpose_row(0, mid_cb=_lw_chunk0)
            for c in range(1, 6):
                nc.sync.dma_start(
                    out=lW_sb[:, 2 * c : 2 * (c + 1), :],
                    in_=lWT_r[:, 2 * c : 2 * (c + 1), :],
                )
            lb_sb = const.tile([P, KT_D], F32, name="lb_sb")
            nc.scalar.dma_start(out=lb_sb, in_=lb_d[:])
            rb_sb = const.tile([P, KT_D], F32, name="rb_sb")
            nc.scalar.dma_start(out=rb_sb, in_=rb_d[:])
            lhwb_sb = const.tile([P, NL, 2 * KT_D], F32, name="lhwb_sb")
            nc.scalar.dma_start(out=lhwb_sb, in_=lhwb_d[:])
            rhwb_sb = const.tile([P, NL, 2 * KT_D], F32, name="rhwb_sb")
            nc.scalar.dma_start(out=rhwb_sb, in_=rhwb_d[:])
            rWT_r = rWT_d[:].rearrange("(kt p) h -> p kt h", p=P)

            def _mid0():
                pend_x[1] = load_transpose_row(1)
                for c in range(6):
                    nc.sync.dma_start(
                        out=rW_sb[:, 2 * c : 2 * (c + 1), :],
                        in_=rWT_r[:, 2 * c : 2 * (c + 1), :],
                    )

            pend_w[0] = window_row(pend_x.pop(0), mid_cb=_mid0)
            nc.sync.dma_start(
                out=lhw_sb, in_=lhw_d[:].rearrange("l (kt p) h -> p l kt h", p=P)
            )
            nc.sync.dma_start(
                out=rhw_sb, in_=rhw_d[:].rearrange("l (kt p) h -> p l kt h", p=P)
            )
            pend_w[1] = window_row(pend_x.pop(1))
            pend_o = {}

            def emit_outputs(r):
                xl, xr = pend_o.pop(r)
                ytoks = [
                    y_pool.tile([P, 2 * D], F32, name="ytok", tag="ytok", bufs=4)
                    for _ in range(TB)
                ]
                out_side(0, xl, ytoks)
                for st in range(TB):
                    nc.sync.dma_start(
                        out=y_d[r, st * P : (st + 1) * P, 0:D],
                        in_=ytoks[st][:, 0:D],
                    )
                out_side(1, xr, ytoks)
                for st in range(TB):
                    nc.sync.dma_start(
                        out=y_d[r, st * P : (st + 1) * P, D:],
                        in_=ytoks[st][:, D:],
                    )

            for r in range(BPC):
                xl, xr = pend_w.pop(r)
                if r + 1 in pend_x:
                    pend_w[r + 1] = window_row(pend_x.pop(r + 1))
                if r + 2 < BPC:
                    pend_x[r + 2] = load_transpose_row(r + 2)
                if r - 1 in pend_o:
                    emit_outputs(r - 1)

                # --- highway stacks; left-side output transposes are
                #     emitted between the two stacks so they hide behind the
                #     right-side matmuls ---
                def out_side(side, xs, ytoks):
                    for st in range(TB):
                        ps_t = psum_pool.tile(
                            [P, KT_D, P], DT, name="ps_tr", tag="tr", bufs=1
                        )
                        for f in range(KT_D):
                            nc.tensor.transpose(
                                out=ps_t[:, f, :],
                                in_=xs[f][:, st * P : (st + 1) * P],
                                identity=ident,
                            )
                        nc.scalar.activation(
                            out=ytoks[st][:, side * D : (side + 1) * D], in_=ps_t,
                            func=mybir.ActivationFunctionType.Copy,
                        )

                xl, xr = highway_pair(xl, xr)
                pend_o[r] = (xl, xr)

            emit_outputs(BPC - 1)

    nc.compile()
    return nc


_CACHE = {}
TRACE = False
LAST_RESULTS = None


def _get_nc():
    if "nc" not in _CACHE:
        _CACHE["nc"] = build_nc()
    return _CACHE["nc"]


def _arr_pad(p):
    # [W, D] -> [P, KT_D, W], partition-major contiguous
    a = np.asarray(p, np.float32).reshape(W, KT_D, P)
    return np.ascontiguousarray(np.transpose(a, (2, 1, 0)))


def _arr_bias(b):
    # [D] -> [P, KT_D]
    return np.ascontiguousarray(np.asarray(b, np.float32).reshape(KT_D, P).T)


def _arr_hwb(b):
    # [NL, 2D] -> [P, NL, 2*KT_D]
    a = np.asarray(b, np.float32).reshape(NL, 2 * KT_D, P)
    return np.ascontiguousarray(np.transpose(a, (2, 0, 1)))


def kernel(
    inputs,
    left_padding,
    right_padding,
    left_W,
    left_b,
    right_W,
    right_b,
    left_hw_W,
    left_hw_b,
    right_hw_W,
    right_hw_b,
):
    global LAST_RESULTS
    f32 = np.float32
    x = np.ascontiguousarray(np.asarray(inputs, dtype=f32))
    shared = {
        "lWT": np.ascontiguousarray(np.asarray(left_W, f32).T),
        "rWT": np.ascontiguousarray(np.asarray(right_W, f32).T),
        "lhwWT": np.ascontiguousarray(
            np.transpose(np.asarray(left_hw_W, f32), (0, 2, 1))
        ),
        "rhwWT": np.ascontiguousarray(
            np.transpose(np.asarray(right_hw_W, f32), (0, 2, 1))
        ),
        "lpadT": _arr_pad(left_padding),
        "rpadT": _arr_pad(right_padding),
        "lb": _arr_bias(left_b),
        "rb": _arr_bias(right_b),
        "lhwb": _arr_hwb(left_hw_b),
        "rhwb": _arr_hwb(right_hw_b),
        "ident": np.eye(P, dtype=f32),
    }
    in_maps = [
        {**shared, "x": np.ascontiguousarray(x[c * BPC : (c + 1) * BPC])}
        for c in range(NCORES)
    ]
    nc = _get_nc()
    res = run_bass_kernel_spmd(nc, in_maps, core_ids=list(range(NCORES)), trace=TRACE)
    LAST_RESULTS = res
    out = np.concatenate([res.results[c]["y"] for c in range(NCORES)], axis=0)
    return (out[None], out)


# revision 35
# speedup vs baseline: 1.0563x; 1.0563x over previous
"""Trainium2 Bass kernel for Bengio03HighwayBiLm.

Reference computation (per batch row b, sequence length S, dim D):
  padded = [left_pad(2), x_b(S), right_pad(2)]           # [S+4, D]
  left_win[t]  = padded[t:t+3]   flattened -> [3D]
  right_win[t] = padded[t+2:t+5] flattened -> [3D]
  left  = relu(left_win  @ left_W.T  + left_b)           # [S, D]
  right = relu(right_win @ right_W.T + right_b)
  2x highway on each side: proj = x @ W_l.T + b_l  ([S, 2D])
     x = sigmoid(proj[:,D:]) * x + (1-sigmoid(proj[:,D:])) * relu(proj[:,:D])
  out = concat([left, right], -1)                        # [S, 2D]

Strategy: data-parallel over the batch dim across 8 NeuronCores (4 rows per
core).  On-chip layout is feature-major ([feature partitions, token free dim])
so the window projection becomes 3 shifted GEMMs accumulated in PSUM and the
highway GEMMs chain with no transposes.  The input is transposed on the PE
(128x128 transpose-mode tiles) on the way in, and the final activations are
transposed back on the way out.  Matmuls run in float32r (full-rate 4-byte
mode at N=512).
"""

import numpy as np

import concourse.bass as bass
import concourse.mybir as mybir
import concourse.tile as tile
from concourse import bacc
from concourse.bass_utils import run_bass_kernel_spmd

B, S, D = 32, 512, 512
W = 2
NL = 2
IN_SZ = D * (W + 1)
NCORES = 8
BPC = B // NCORES            # batch rows per core
P = 128
KT_D = D // P                # 4 feature tiles of the model dim
KT_IN = IN_SZ // P           # 12 contraction tiles of the window GEMM
TB = S // P                  # 4 token tiles per row

F32 = mybir.dt.float32
DT = mybir.dt.float32r       # matmul I/O dtype (4-byte, full PE rate at N>=256)

RELU = mybir.ActivationFunctionType.Relu
SIGMOID = mybir.ActivationFunctionType.Sigmoid


def build_nc():
    nc = bacc.Bacc("TRN2", target_bir_lowering=False, debug=False)

    x_d = nc.dram_tensor("x", [BPC, S, D], DT, kind="ExternalInput")
    lWT_d = nc.dram_tensor("lWT", [IN_SZ, D], DT, kind="ExternalInput")
    rWT_d = nc.dram_tensor("rWT", [IN_SZ, D], DT, kind="ExternalInput")
    lhw_d = nc.dram_tensor("lhwWT", [NL, D, 2 * D], DT, kind="ExternalInput")
    rhw_d = nc.dram_tensor("rhwWT", [NL, D, 2 * D], DT, kind="ExternalInput")
    lpad_d = nc.dram_tensor("lpadT", [P, KT_D, W], DT, kind="ExternalInput")
    rpad_d = nc.dram_tensor("rpadT", [P, KT_D, W], DT, kind="ExternalInput")
    lb_d = nc.dram_tensor("lb", [P, KT_D], F32, kind="ExternalInput")
    rb_d = nc.dram_tensor("rb", [P, KT_D], F32, kind="ExternalInput")
    lhwb_d = nc.dram_tensor("lhwb", [P, NL, 2 * KT_D], F32, kind="ExternalInput")
    rhwb_d = nc.dram_tensor("rhwb", [P, NL, 2 * KT_D], F32, kind="ExternalInput")
    ident_d = nc.dram_tensor("ident", [P, P], DT, kind="ExternalInput")
    y_d = nc.dram_tensor("y", [BPC, S, 2 * D], F32, kind="ExternalOutput")

    with tile.TileContext(nc) as tc:
        with (
            tc.tile_pool(name="const", bufs=1) as const,
            tc.tile_pool(name="xtok", bufs=2) as xtok_pool,
            tc.tile_pool(name="xpad", bufs=2) as xpad_pool,
            tc.tile_pool(name="act", bufs=9) as act_pool,
            tc.tile_pool(name="ew", bufs=3) as ew_pool,
            tc.tile_pool(name="yout", bufs=5) as y_pool,
            tc.tile_pool(name="psum", bufs=6, space="PSUM") as psum_pool,
        ):
            ident = const.tile([P, P], DT, name="ident")
            nc.sync.dma_start(out=ident, in_=ident_d[:])

            # weights stream on the scalar HWDGE queue, concurrently with the
            # input loads on the sync queue, in first-use order
            lW_sb = const.tile([P, KT_IN, D], DT, name="lW_sb")
            rW_sb = const.tile([P, KT_IN, D], DT, name="rW_sb")
            lhw_sb = const.tile([P, NL, KT_D, 2 * D], DT, name="lhw_sb")
            rhw_sb = const.tile([P, NL, KT_D, 2 * D], DT, name="rhw_sb")
            lpad_sb = const.tile([P, KT_D, W], DT, name="lpad_sb")
            nc.scalar.dma_start(out=lpad_sb, in_=lpad_d[:])
            rpad_sb = const.tile([P, KT_D, W], DT, name="rpad_sb")
            nc.scalar.dma_start(out=rpad_sb, in_=rpad_d[:])

            def window_gemm(xpads, W_sb, b_sb, j_off, tag):
                """relu(win @ W.T + b): 12 accumulated MMs per output ftile,
                k-outer so each weight k-tile is consumed once, as it lands."""
                pss = [
                    psum_pool.tile([P, S], F32, name="ps_mm", tag="mm", bufs=7)
                    for _ in range(KT_D)
                ]
                for j in range(W + 1):
                    for k in range(KT_D):
                        ki = j * KT_D + k
                        for f in range(KT_D):
                            nc.tensor.matmul(
                                out=pss[f],
                                lhsT=W_sb[:, ki, f * P : (f + 1) * P],
                                rhs=xpads[k][:, j + j_off : j + j_off + S],
                                start=(ki == 0),
                                stop=(ki == KT_IN - 1),
                            )
                out_tiles = []
                for f in range(KT_D):
                    t = act_pool.tile([P, S], DT, name=f"x_{tag}", tag=f"x_{tag}")
                    nc.scalar.activation(
                        out=t, in_=pss[f], func=RELU, bias=b_sb[:, f : f + 1],
                        scale=1.0,
                    )
                    out_tiles.append(t)
                return out_tiles

            def highway_pair(xl_tiles, xr_tiles):
                """Both highway stacks, layer-interleaved across sides so each
                side's ACT/DVE drain chain hides behind the other side's
                matmuls."""
                sides = {"l": (xl_tiles, lhw_sb, lhwb_sb), "r": (xr_tiles, rhw_sb, rhwb_sb)}
                for l in range(NL):
                    for tag in ("l", "r"):
                        x_tiles, hw_sb, hwb_sb = sides[tag]
                        new_x = []
                        for f in range(KT_D):
                            ps_n = psum_pool.tile([P, S], F32, name="ps_mm", tag="mm", bufs=7)
                            for k in range(KT_D):
                                nc.tensor.matmul(
                                    out=ps_n,
                                    lhsT=hw_sb[:, l, k, f * P : (f + 1) * P],
                                    rhs=x_tiles[k],
                                    start=(k == 0),
                                    stop=(k == KT_D - 1),
                                )
                            nl_t = ew_pool.tile([P, S], DT, name="nl_t", tag="nl")
                            nc.scalar.activation(
                                out=nl_t, in_=ps_n, func=RELU,
                                bias=hwb_sb[:, l, f : f + 1], scale=1.0,
                            )
                            ps_g = psum_pool.tile([P, S], F32, name="ps_mm", tag="mm", bufs=7)
                            for k in range(KT_D):
                                nc.tensor.matmul(
                                    out=ps_g,
                                    lhsT=hw_sb[:, l, k, D + f * P : D + (f + 1) * P],
                                    rhs=x_tiles[k],
                                    start=(k == 0),
                                    stop=(k == KT_D - 1),
                                )
                            g_t = ew_pool.tile([P, S], DT, name="g_t", tag="g")
                            nc.scalar.activation(
                                out=g_t, in_=ps_g, func=SIGMOID,
                                bias=hwb_sb[:, l, KT_D + f : KT_D + f + 1], scale=1.0,
                            )
                            # y = g*x + (1-g)*nl = nl + g*(x - nl)
                            d_t = ew_pool.tile([P, S], DT, name="d_t", tag="d")
                            nc.vector.tensor_sub(out=d_t, in0=x_tiles[f], in1=nl_t)
                            nc.vector.tensor_mul(out=d_t, in0=d_t, in1=g_t)
                            xn = act_pool.tile([P, S], DT, name=f"x_{tag}", tag=f"x_{tag}")
                            nc.vector.tensor_add(out=xn, in0=d_t, in1=nl_t)
                            new_x.append(xn)
                        sides[tag] = (new_x, hw_sb, hwb_sb)
                return sides["l"][0], sides["r"][0]

            def load_transpose_row(r, mid_cb=None):
                """DMA row r token-major and PE-transpose into a padded
                feature-major tile [P, KT_D, S+2W]."""
                xp = xpad_pool.tile([P, KT_D, S + 2 * W], DT, name="xp", tag="xp")
                nc.vector.tensor_copy(out=xp[:, :, 0:W], in_=lpad_sb)
                nc.vector.tensor_copy(out=xp[:, :, W + S :], in_=rpad_sb)
                for st in range(TB):
                    x_tok = xtok_pool.tile([P, D], DT, name="x_tok", tag="x_tok")
                    nc.sync.dma_start(out=x_tok, in_=x_d[r, st * P : (st + 1) * P, :])
                    if st == 1 and mid_cb is not None:
                        mid_cb()
                    ps_t = psum_pool.tile([P, KT_D, P], DT, name="ps_tr", tag="tr", bufs=1)
                    for k in range(KT_D):
                        nc.tensor.transpose(
                            out=ps_t[:, k, :], in_=x_tok[:, k * P : (k + 1) * P],
                            identity=ident,
                        )
                    nc.vector.tensor_copy(
                        out=xp[:, :, W + st * P : W + (st + 1) * P], in_=ps_t
                    )
                return [xp[:, k, :] for k in range(KT_D)]

            # 2-deep software pipeline over rows: window GEMMs run one row
            # ahead of the highway stack so the highway-weight DMA stream has
            # PE work to hide behind; input transposes run a row ahead of the
            # window GEMMs.
            def window_row(xpads, mid_cb=None):
                xl = window_gemm(xpads, lW_sb, lb_sb, 0, "l")
                if mid_cb is not None:
                    mid_cb()
                xr = window_gemm(xpads, rW_sb, rb_sb, W, "r")
                return xl, xr

            pend_x = {}   # r -> xpads
            pend_w = {}   # r -> (xl, xr)
            lWT_r = lWT_d[:].rearrange("(kt p) h -> p kt h", p=P)

            def _lw_chunk0():
                nc.sync.dma_start(out=lW_sb[:, 0:2, :], in_=lWT_r[:, 0:2, :])

            pend_x[0] = load_transpose_row(0, mid_cb=_lw_chunk0)
            for c in range(1, 6):
                nc.sync.dma_start(
                    out=lW_sb[:, 2 * c : 2 * (c + 1), :],
                    in_=lWT_r[:, 2 * c : 2 * (c + 1), :],
                )
            lb_sb = const.tile([P, KT_D], F32, name="lb_sb")
            nc.scalar.dma_start(out=lb_sb, in_=lb_d[:])
            rb_sb = const.tile([P, KT_D], F32, name="rb_sb")
            nc.scalar.dma_start(out=rb_sb, in_=rb_d[:])
            lhwb_sb = const.tile([P, NL, 2 * KT_D], F32, name="lhwb_sb")
            nc.scalar.dma_start(out=lhwb_sb, in_=lhwb_d[:])
            rhwb_sb = const.tile([P, NL, 2 * KT_D], F32, name="rhwb_sb")
            nc.scalar.dma_start(out=rhwb_sb, in_=rhwb_d[:])
            rWT_r = rWT_d[:].rearrange("(kt p) h -> p kt h", p=P)

            def _mid0():
                pend_x[1] = load_transpose_row(1)
                for c in range(6):
                    nc.sync.dma_start(
                        out=rW_sb[:, 2 * c : 2 * (c + 1), :],
                        in_=rWT_r[:, 2 * c : 2 * (c + 1), :],
                    )

            pend_w[0] = window_row(pend_x.pop(0), mid_cb=_mid0)
            nc.sync.dma_start(
                out=lhw_sb, in_=lhw_d[:].rearrange("l (kt p) h -> p l kt h", p=P)
            )
            nc.sync.dma_start(
                out=rhw_sb, in_=rhw_d[:].rearrange("l (kt p) h -> p l kt h", p=P)
            )
            pend_w[1] = window_row(pend_x.pop(1))
            pend_o = {}

            def emit_outputs(r):
                xl, xr = pend_o.pop(r)
                ytoks = [
                    y_pool.tile([P, 2 * D], F32, name="ytok", tag="ytok", bufs=5)
                    for _ in range(TB)
                ]
                out_side(0, xl, ytoks)
                for st in range(TB):
                    nc.sync.dma_start(
                        out=y_d[r, st * P : (st + 1) * P, 0:D],
                        in_=ytoks[st][:, 0:D],
                    )
                out_side(1, xr, ytoks)
                for st in range(TB):
                    nc.sync.dma_start(
                        out=y_d[r, st * P : (st + 1) * P, D:],
                        in_=ytoks[st][:, D:],
                    )

            for r in range(BPC):
                xl, xr = pend_w.pop(r)
                if r + 1 in pend_x:
                    pend_w[r + 1] = window_row(pend_x.pop(r + 1))
                if r + 2 < BPC:
                    pend_x[r + 2] = load_transpose_row(r + 2)
                if r - 1 in pend_o:
                    emit_outputs(r - 1)

                # --- highway stacks; left-side output transposes are
                #     emitted between the two stacks so they hide behind the
                #     right-side matmuls ---
                def out_side(side, xs, ytoks):
                    for st in range(TB):
                        ps_t = psum_pool.tile(
                            [P, KT_D, P], DT, name="ps_tr", tag="tr", bufs=1
                        )
                        for f in range(KT_D):
                            nc.tensor.transpose(
                                out=ps_t[:, f, :],
                                in_=xs[f][:, st * P : (st + 1) * P],
                                identity=ident,
                            )
                        nc.scalar.activation(
                            out=ytoks[st][:, side * D : (side + 1) * D], in_=ps_t,
                            func=mybir.ActivationFunctionType.Copy,
                        )

                xl, xr = highway_pair(xl, xr)
                pend_o[r] = (xl, xr)

            emit_outputs(BPC - 1)

    nc.compile()
    return nc


_CACHE = {}
TRACE = False
LAST_RESULTS = None


def _get_nc():
    if "nc" not in _CACHE:
        _CACHE["nc"] = build_nc()
    return _CACHE["nc"]


def _arr_pad(p):
    # [W, D] -> [P, KT_D, W], partition-major contiguous
    a = np.asarray(p, np.float32).reshape(W, KT_D, P)
    return np.ascontiguousarray(np.transpose(a, (2, 1, 0)))


def _arr_bias(b):
    # [D] -> [P, KT_D]
    return np.ascontiguousarray(np.asarray(b, np.float32).reshape(KT_D, P).T)


def _arr_hwb(b):
    # [NL, 2D] -> [P, NL, 2*KT_D]
    a = np.asarray(b, np.float32).reshape(NL, 2 * KT_D, P)
    return np.ascontiguousarray(np.transpose(a, (2, 0, 1)))


def kernel(
    inputs,
    left_padding,
    right_padding,
    left_W,
    left_b,
    right_W,
    right_b,
    left_hw_W,
    left_hw_b,
    right_hw_W,
    right_hw_b,
):
    global LAST_RESULTS
    f32 = np.float32
    x = np.ascontiguousarray(np.asarray(inputs, dtype=f32))
    shared = {
        "lWT": np.ascontiguousarray(np.asarray(left_W, f32).T),
        "rWT": np.ascontiguousarray(np.asarray(right_W, f32).T),
        "lhwWT": np.ascontiguousarray(
            np.transpose(np.asarray(left_hw_W, f32), (0, 2, 1))
        ),
        "rhwWT": np.ascontiguousarray(
            np.transpose(np.asarray(right_hw_W, f32), (0, 2, 1))
        ),
        "lpadT": _arr_pad(left_padding),
        "rpadT": _arr_pad(right_padding),
        "lb": _arr_bias(left_b),
        "rb": _arr_bias(right_b),
        "lhwb": _arr_hwb(left_hw_b),
        "rhwb": _arr_hwb(right_hw_b),
        "ident": np.eye(P, dtype=f32),
    }
    in_maps = [
        {**shared, "x": np.ascontiguousarray(x[c * BPC : (c + 1) * BPC])}
        for c in range(NCORES)
    ]
    nc = _get_nc()
    res = run_bass_kernel_spmd(nc, in_maps, core_ids=list(range(NCORES)), trace=TRACE)
    LAST_RESULTS = res
    out = np.concatenate([res.results[c]["y"] for c in range(NCORES)], axis=0)
    return (out[None], out)


# revision 36
# speedup vs baseline: 1.0696x; 1.0125x over previous
"""Trainium2 Bass kernel for Bengio03HighwayBiLm.

Reference computation (per batch row b, sequence length S, dim D):
  padded = [left_pad(2), x_b(S), right_pad(2)]           # [S+4, D]
  left_win[t]  = padded[t:t+3]   flattened -> [3D]
  right_win[t] = padded[t+2:t+5] flattened -> [3D]
  left  = relu(left_win  @ left_W.T  + left_b)           # [S, D]
  right = relu(right_win @ right_W.T + right_b)
  2x highway on each side: proj = x @ W_l.T + b_l  ([S, 2D])
     x = sigmoid(proj[:,D:]) * x + (1-sigmoid(proj[:,D:])) * relu(proj[:,:D])
  out = concat([left, right], -1)                        # [S, 2D]

Strategy: data-parallel over the batch dim across 8 NeuronCores (4 rows per
core).  On-chip layout is feature-major ([feature partitions, token free dim])
so the window projection becomes 3 shifted GEMMs accumulated in PSUM and the
highway GEMMs chain with no transposes.  The input is transposed on the PE
(128x128 transpose-mode tiles) on the way in, and the final activations are
transposed back on the way out.  Matmuls run in float32r (full-rate 4-byte
mode at N=512).
"""

import numpy as np

import concourse.bass as bass
import concourse.mybir as mybir
import concourse.tile as tile
from concourse import bacc
from concourse.bass_utils import run_bass_kernel_spmd

B, S, D = 32, 512, 512
W = 2
NL = 2
IN_SZ = D * (W + 1)
NCORES = 8
BPC = B // NCORES            # batch rows per core
P = 128
KT_D = D // P                # 4 feature tiles of the model dim
KT_IN = IN_SZ // P           # 12 contraction tiles of the window GEMM
TB = S // P                  # 4 token tiles per row

F32 = mybir.dt.float32
DT = mybir.dt.float32r       # matmul I/O dtype (4-byte, full PE rate at N>=256)

RELU = mybir.ActivationFunctionType.Relu
SIGMOID = mybir.ActivationFunctionType.Sigmoid


def build_nc():
    nc = bacc.Bacc("TRN2", target_bir_lowering=False, debug=False)

    x_d = nc.dram_tensor("x", [BPC, S, D], DT, kind="ExternalInput")
    lWT_d = nc.dram_tensor("lWT", [IN_SZ, D], DT, kind="ExternalInput")
    rWT_d = nc.dram_tensor("rWT", [IN_SZ, D], DT, kind="ExternalInput")
    lhw_d = nc.dram_tensor("lhwWT", [NL, D, 2 * D], DT, kind="ExternalInput")
    rhw_d = nc.dram_tensor("rhwWT", [NL, D, 2 * D], DT, kind="ExternalInput")
    lpad_d = nc.dram_tensor("lpadT", [P, KT_D, W], DT, kind="ExternalInput")
    rpad_d = nc.dram_tensor("rpadT", [P, KT_D, W], DT, kind="ExternalInput")
    lb_d = nc.dram_tensor("lb", [P, KT_D], F32, kind="ExternalInput")
    rb_d = nc.dram_tensor("rb", [P, KT_D], F32, kind="ExternalInput")
    lhwb_d = nc.dram_tensor("lhwb", [P, NL, 2 * KT_D], F32, kind="ExternalInput")
    rhwb_d = nc.dram_tensor("rhwb", [P, NL, 2 * KT_D], F32, kind="ExternalInput")
    ident_d = nc.dram_tensor("ident", [P, P], DT, kind="ExternalInput")
    y_d = nc.dram_tensor("y", [BPC, S, 2 * D], F32, kind="ExternalOutput")

    with tile.TileContext(nc) as tc:
        with (
            tc.tile_pool(name="const", bufs=1) as const,
            tc.tile_pool(name="xtok", bufs=2) as xtok_pool,
            tc.tile_pool(name="xpad", bufs=2) as xpad_pool,
            tc.tile_pool(name="act", bufs=9) as act_pool,
            tc.tile_pool(name="ew", bufs=3) as ew_pool,
            tc.tile_pool(name="yout", bufs=4) as y_pool,
            tc.tile_pool(name="psum", bufs=6, space="PSUM") as psum_pool,
        ):
            ident = const.tile([P, P], DT, name="ident")
            nc.sync.dma_start(out=ident, in_=ident_d[:])

            # weights stream on the scalar HWDGE queue, concurrently with the
            # input loads on the sync queue, in first-use order
            lW_sb = const.tile([P, KT_IN, D], DT, name="lW_sb")
            rW_sb = const.tile([P, KT_IN, D], DT, name="rW_sb")
            lhw_sb = const.tile([P, NL, KT_D, 2 * D], DT, name="lhw_sb")
            rhw_sb = const.tile([P, NL, KT_D, 2 * D], DT, name="rhw_sb")
            lpad_sb = const.tile([P, KT_D, W], DT, name="lpad_sb")
            nc.scalar.dma_start(out=lpad_sb, in_=lpad_d[:])
            rpad_sb = const.tile([P, KT_D, W], DT, name="rpad_sb")
            nc.scalar.dma_start(out=rpad_sb, in_=rpad_d[:])

            def window_gemm(xpads, W_sb, b_sb, j_off, tag):
                """relu(win @ W.T + b): 12 accumulated MMs per output ftile,
                k-outer so each weight k-tile is consumed once, as it lands."""
                pss = [
                    psum_pool.tile([P, S], F32, name="ps_mm", tag="mm", bufs=7)
                    for _ in range(KT_D)
                ]
                for j in range(W + 1):
                    for k in range(KT_D):
                        ki = j * KT_D + k
                        for f in range(KT_D):
                            nc.tensor.matmul(
                                out=pss[f],
                                lhsT=W_sb[:, ki, f * P : (f + 1) * P],
                                rhs=xpads[k][:, j + j_off : j + j_off + S],
                                start=(ki == 0),
                                stop=(ki == KT_IN - 1),
                            )
                out_tiles = []
                for f in range(KT_D):
                    t = act_pool.tile([P, S], DT, name=f"x_{tag}", tag=f"x_{tag}")
                    nc.scalar.activation(
                        out=t, in_=pss[f], func=RELU, bias=b_sb[:, f : f + 1],
                        scale=1.0,
                    )
                    out_tiles.append(t)
                return out_tiles

            def highway_pair(xl_tiles, xr_tiles):
                """Both highway stacks, layer-interleaved across sides so each
                side's ACT/DVE drain chain hides behind the other side's
                matmuls."""
                sides = {"l": (xl_tiles, lhw_sb, lhwb_sb), "r": (xr_tiles, rhw_sb, rhwb_sb)}
                for l in range(NL):
                    for tag in ("l", "r"):
                        x_tiles, hw_sb, hwb_sb = sides[tag]
                        new_x = []
                        for f in range(KT_D):
                            ps_n = psum_pool.tile([P, S], F32, name="ps_mm", tag="mm", bufs=7)
                            for k in range(KT_D):
                                nc.tensor.matmul(
                                    out=ps_n,
                                    lhsT=hw_sb[:, l, k, f * P : (f + 1) * P],
                                    rhs=x_tiles[k],
                                    start=(k == 0),
                                    stop=(k == KT_D - 1),
                                )
                            nl_t = ew_pool.tile([P, S], DT, name="nl_t", tag="nl", bufs=4)
                            nc.scalar.activation(
                                out=nl_t, in_=ps_n, func=RELU,
                                bias=hwb_sb[:, l, f : f + 1], scale=1.0,
                            )
                            ps_g = psum_pool.tile([P, S], F32, name="ps_mm", tag="mm", bufs=7)
                            for k in range(KT_D):
                                nc.tensor.matmul(
                                    out=ps_g,
                                    lhsT=hw_sb[:, l, k, D + f * P : D + (f + 1) * P],
                                    rhs=x_tiles[k],
                                    start=(k == 0),
                                    stop=(k == KT_D - 1),
                                )
                            g_t = ew_pool.tile([P, S], DT, name="g_t", tag="g", bufs=4)
                            nc.scalar.activation(
                                out=g_t, in_=ps_g, func=SIGMOID,
                                bias=hwb_sb[:, l, KT_D + f : KT_D + f + 1], scale=1.0,
                            )
                            # y = g*x + (1-g)*nl = nl + g*(x - nl)
                            d_t = ew_pool.tile([P, S], DT, name="d_t", tag="d")
                            nc.vector.tensor_sub(out=d_t, in0=x_tiles[f], in1=nl_t)
                            nc.vector.tensor_mul(out=d_t, in0=d_t, in1=g_t)
                            xn = act_pool.tile([P, S], DT, name=f"x_{tag}", tag=f"x_{tag}")
                            nc.vector.tensor_add(out=xn, in0=d_t, in1=nl_t)
                            new_x.append(xn)
                        sides[tag] = (new_x, hw_sb, hwb_sb)
                return sides["l"][0], sides["r"][0]

            def load_transpose_row(r, mid_cb=None):
                """DMA row r token-major and PE-transpose into a padded
                feature-major tile [P, KT_D, S+2W]."""
                xp = xpad_pool.tile([P, KT_D, S + 2 * W], DT, name="xp", tag="xp")
                nc.vector.tensor_copy(out=xp[:, :, 0:W], in_=lpad_sb)
                nc.vector.tensor_copy(out=xp[:, :, W + S :], in_=rpad_sb)
                for st in range(TB):
                    x_tok = xtok_pool.tile([P, D], DT, name="x_tok", tag="x_tok")
                    nc.sync.dma_start(out=x_tok, in_=x_d[r, st * P : (st + 1) * P, :])
                    if st == 1 and mid_cb is not None:
                        mid_cb()
                    ps_t = psum_pool.tile([P, KT_D, P], DT, name="ps_tr", tag="tr", bufs=1)
                    for k in range(KT_D):
                        nc.tensor.transpose(
                            out=ps_t[:, k, :], in_=x_tok[:, k * P : (k + 1) * P],
                            identity=ident,
                        )
                    nc.vector.tensor_copy(
                        out=xp[:, :, W + st * P : W + (st + 1) * P], in_=ps_t
                    )
                return [xp[:, k, :] for k in range(KT_D)]

            # 2-deep software pipeline over rows: window GEMMs run one row
            # ahead of the highway stack so the highway-weight DMA stream has
            # PE work to hide behind; input transposes run a row ahead of the
            # window GEMMs.
            def window_row(xpads, mid_cb=None):
                xl = window_gemm(xpads, lW_sb, lb_sb, 0, "l")
                if mid_cb is not None:
                    mid_cb()
                xr = window_gemm(xpads, rW_sb, rb_sb, W, "r")
                return xl, xr

            pend_x = {}   # r -> xpads
            pend_w = {}   # r -> (xl, xr)
            lWT_r = lWT_d[:].rearrange("(kt p) h -> p kt h", p=P)

            def _lw_chunk0():
                nc.sync.dma_start(out=lW_sb[:, 0:2, :], in_=lWT_r[:, 0:2, :])

            pend_x[0] = load_transpose_row(0, mid_cb=_lw_chunk0)
            for c in range(1, 6):
                nc.sync.dma_start(
                    out=lW_sb[:, 2 * c : 2 * (c + 1), :],
                    in_=lWT_r[:, 2 * c : 2 * (c + 1), :],
                )
            lb_sb = const.tile([P, KT_D], F32, name="lb_sb")
            nc.scalar.dma_start(out=lb_sb, in_=lb_d[:])
            rb_sb = const.tile([P, KT_D], F32, name="rb_sb")
            nc.scalar.dma_start(out=rb_sb, in_=rb_d[:])
            lhwb_sb = const.tile([P, NL, 2 * KT_D], F32, name="lhwb_sb")
            nc.scalar.dma_start(out=lhwb_sb, in_=lhwb_d[:])
            rhwb_sb = const.tile([P, NL, 2 * KT_D], F32, name="rhwb_sb")
            nc.scalar.dma_start(out=rhwb_sb, in_=rhwb_d[:])
            rWT_r = rWT_d[:].rearrange("(kt p) h -> p kt h", p=P)

            def _mid0():
                pend_x[1] = load_transpose_row(1)
                for c in range(6):
                    nc.sync.dma_start(
                        out=rW_sb[:, 2 * c : 2 * (c + 1), :],
                        in_=rWT_r[:, 2 * c : 2 * (c + 1), :],
                    )

            pend_w[0] = window_row(pend_x.pop(0), mid_cb=_mid0)
            nc.sync.dma_start(
                out=lhw_sb, in_=lhw_d[:].rearrange("l (kt p) h -> p l kt h", p=P)
            )
            nc.sync.dma_start(
                out=rhw_sb, in_=rhw_d[:].rearrange("l (kt p) h -> p l kt h", p=P)
            )
            pend_w[1] = window_row(pend_x.pop(1))
            pend_o = {}

            def emit_outputs(r):
                xl, xr = pend_o.pop(r)
                ytoks = [
                    y_pool.tile([P, 2 * D], F32, name="ytok", tag="ytok", bufs=4)
                    for _ in range(TB)
                ]
                out_side(0, xl, ytoks)
                for st in range(TB):
                    nc.sync.dma_start(
                        out=y_d[r, st * P : (st + 1) * P, 0:D],
                        in_=ytoks[st][:, 0:D],
                    )
                out_side(1, xr, ytoks)
                for st in range(TB):
                    nc.sync.dma_start(
                        out=y_d[r, st * P : (st + 1) * P, D:],
                        in_=ytoks[st][:, D:],
                    )

            for r in range(BPC):
                xl, xr = pend_w.pop(r)
                if r + 1 in pend_x:
                    pend_w[r + 1] = window_row(pend_x.pop(r + 1))
                if r + 2 < BPC:
                    pend_x[r + 2] = load_transpose_row(r + 2)
                if r - 1 in pend_o:
                    emit_outputs(r - 1)

                # --- highway stacks; left-side output transposes are
                #     emitted between the two stacks so they hide behind the
                #     right-side matmuls ---
                def out_side(side, xs, ytoks):
                    for st in range(TB):
                        ps_t = psum_pool.tile(
                            [P, KT_D, P], DT, name="ps_tr", tag="tr", bufs=1
                        )
                        for f in range(KT_D):
                            nc.tensor.transpose(
                                out=ps_t[:, f, :],
                                in_=xs[f][:, st * P : (st + 1) * P],
                                identity=ident,
                            )
                        nc.scalar.activation(
                            out=ytoks[st][:, side * D : (side + 1) * D], in_=ps_t,
                            func=mybir.ActivationFunctionType.Copy,
                        )

                xl, xr = highway_pair(xl, xr)
                pend_o[r] = (xl, xr)

            emit_outputs(BPC - 1)

    nc.compile()
    return nc


_CACHE = {}
TRACE = False
LAST_RESULTS = None


def _get_nc():
    if "nc" not in _CACHE:
        _CACHE["nc"] = build_nc()
    return _CACHE["nc"]


def _arr_pad(p):
    # [W, D] -> [P, KT_D, W], partition-major contiguous
    a = np.asarray(p, np.float32).reshape(W, KT_D, P)
    return np.ascontiguousarray(np.transpose(a, (2, 1, 0)))


def _arr_bias(b):
    # [D] -> [P, KT_D]
    return np.ascontiguousarray(np.asarray(b, np.float32).reshape(KT_D, P).T)


def _arr_hwb(b):
    # [NL, 2D] -> [P, NL, 2*KT_D]
    a = np.asarray(b, np.float32).reshape(NL, 2 * KT_D, P)
    return np.ascontiguousarray(np.transpose(a, (2, 0, 1)))


def kernel(
    inputs,
    left_padding,
    right_padding,
    left_W,
    left_b,
    right_W,
    right_b,
    left_hw_W,
    left_hw_b,
    right_hw_W,
    right_hw_b,
):
    global LAST_RESULTS
    f32 = np.float32
    x = np.ascontiguousarray(np.asarray(inputs, dtype=f32))
    shared = {
        "lWT": np.ascontiguousarray(np.asarray(left_W, f32).T),
        "rWT": np.ascontiguousarray(np.asarray(right_W, f32).T),
        "lhwWT": np.ascontiguousarray(
            np.transpose(np.asarray(left_hw_W, f32), (0, 2, 1))
        ),
        "rhwWT": np.ascontiguousarray(
            np.transpose(np.asarray(right_hw_W, f32), (0, 2, 1))
        ),
        "lpadT": _arr_pad(left_padding),
        "rpadT": _arr_pad(right_padding),
        "lb": _arr_bias(left_b),
        "rb": _arr_bias(right_b),
        "lhwb": _arr_hwb(left_hw_b),
        "rhwb": _arr_hwb(right_hw_b),
        "ident": np.eye(P, dtype=f32),
    }
    in_maps = [
        {**shared, "x": np.ascontiguousarray(x[c * BPC : (c + 1) * BPC])}
        for c in range(NCORES)
    ]
    nc = _get_nc()
    res = run_bass_kernel_spmd(nc, in_maps, core_ids=list(range(NCORES)), trace=TRACE)
    LAST_RESULTS = res
    out = np.concatenate([res.results[c]["y"] for c in range(NCORES)], axis=0)
    return (out[None], out)
